# revision 7
# baseline (speedup 1.0000x reference)
"""CLIP loss kernel for trn2, 8 NeuronCores, data-parallel over the batch dim.

Strategy (per core c of 8, SPMD) — no collectives:
  The host pre-normalizes both modalities (x / max(|x|, 1e-3), matching the
  reference), pre-transposes them into PE lhsT/rhs layout, scales by 16 and
  casts to fp8e4m3 (entries of unit rows are <= 1, so x16 uses fp8's range).
  Each core receives its own 1024 img rows (transposed, [128, 4, 1024]) plus
  the FULL transposed spec matrix ([128, 4, 8192]) — replicating 4 MB of fp8
  to every core replaces the AllGather + mesh barrier of the collective
  formulation, which otherwise serializes ~60us at the head of the kernel.

  Device work per core is a single pipeline:
    logits block [1024, 8192] = imgT.T @ specT, fp8 DoubleRow matmuls
    (K=256 per pass, PSUM f32, [128, 2048] tiles), then ACT Exp with
    scale = logit_scale/256 (the 16x16 fp8 prescale cancels); accum_out
    yields row-sums of exp for free; the exp tile (bf16, SBUF) accumulates
    into racc[128, 8192] (DVE add) = column partial sums stratified by
    partition.
  Host: log/sum of row sums and column sums (O(N) numpy), diagonal term
  computed directly on the host in f64 -> scalar loss.
"""

import os
from contextlib import ExitStack

import numpy as np

import concourse.bass as bass
import concourse.mybir as mybir
from concourse import bacc, tile
from concourse.bass_utils import run_bass_kernel_spmd

N, D, C = 8192, 512, 8
NL = N // C  # 1024 local rows per core
P = 128
T = NL // P  # 8 [128 row] tiles per core
KC = D // P  # 4 contraction chunks of 128
G = 4        # column groups
GW = N // G  # 2048 columns per group

f32 = mybir.dt.float32
bf16 = mybir.dt.bfloat16
fp8 = mybir.dt.float8e4
FA = mybir.ActivationFunctionType

# operands are pre-scaled by 16 on the host to center fp8's dynamic range;
# the matmul result is 256x too big, compensated in the exp scale.
FP8_PRESCALE = 16.0

_cache: dict = {}


def _build(scale: float):
    nc = bacc.Bacc("TRN2", target_bir_lowering=False, debug=False, num_devices=C)
    imgT_d = nc.dram_tensor("imgT", [P, KC, NL], fp8, kind="ExternalInput")
    specT_d = nc.dram_tensor("specT", [P, KC, N], fp8, kind="ExternalInput")
    rowsum_o = nc.dram_tensor("rowsum", [P, T], f32, kind="ExternalOutput")
    racc_o = nc.dram_tensor("racc_o", [P, N], bf16, kind="ExternalOutput")

    with tile.TileContext(nc) as tc, ExitStack() as ctx:
        const = ctx.enter_context(tc.tile_pool(name="const", bufs=1))
        pers = ctx.enter_context(tc.tile_pool(name="pers", bufs=1))
        ps = ctx.enter_context(tc.tile_pool(name="ps", bufs=2, space="PSUM"))
        ep = ctx.enter_context(tc.tile_pool(name="e", bufs=4))

        imgT = pers.tile([P, KC, NL], fp8, name="imgT")
        specT = pers.tile([P, KC, N], fp8, name="specT")
        racc = pers.tile([P, N], bf16, name="racc")
        rowacc = pers.tile([P, T, G], f32, name="rowacc")
        rows = pers.tile([P, T], f32, name="rows")

        # preload the exp activation table while the input DMAs run
        warm = const.tile([P, 1], f32, name="actwarm")
        nc.vector.memset(warm, 1.0)
        nc.scalar.activation(warm, warm, FA.Exp)

        # input DMAs on two trigger engines so img and spec stream
        # concurrently; group 0 first so the (g=0, m=0) matmuls start earliest.
        nc.scalar.dma_start(imgT[:, :, 0:P], imgT_d.ap()[:, :, 0:P])
        nc.scalar.dma_start(imgT[:, :, P:NL], imgT_d.ap()[:, :, P:NL])
        for g in range(G):
            nc.sync.dma_start(
                specT[:, :, GW * g : GW * (g + 1)],
                specT_d.ap()[:, :, GW * g : GW * (g + 1)],
            )

        # main loop: logits block, exp, row/col accumulation
        with nc.allow_low_precision("bf16 exp-sum accumulation, <1e-3 on loss"):
            for g in range(G):
                gsl = racc[:, GW * g : GW * (g + 1)]
                for m in range(T):
                    pm = ps.tile([P, GW], f32, tag="mm")
                    # fp8 DoubleRow: each matmul contracts 2 k-chunks (K=256)
                    for q in range(KC // 2):
                        for ns in range(GW // 512):
                            cs = slice(GW * g + 512 * ns, GW * g + 512 * (ns + 1))
                            nc.tensor.matmul(
                                pm[:, 512 * ns : 512 * (ns + 1)],
                                imgT[:, 2 * q : 2 * q + 2, P * m : P * (m + 1)],
                                specT[:, 2 * q : 2 * q + 2, cs],
                                start=(q == 0),
                                stop=(q == KC // 2 - 1),
                                perf_mode=mybir.MatmulPerfMode.DoubleRow,
                            )
                    e = ep.tile([P, GW], bf16, tag="e")
                    nc.scalar.activation(
                        e, pm, FA.Exp,
                        scale=scale / (FP8_PRESCALE * FP8_PRESCALE),
                        accum_out=rowacc[:, m, g : g + 1],
                    )
                    if m == 0:
                        nc.vector.tensor_copy(gsl, e)
                    elif m == T - 1 and g == G - 1:
                        # final tile: halve the add so the write-out overlaps
                        for h in range(2):
                            hs = slice(GW // 2 * h, GW // 2 * (h + 1))
                            nc.vector.tensor_add(
                                out=gsl[:, hs], in0=gsl[:, hs], in1=e[:, hs]
                            )
                            nc.sync.dma_start(
                                racc_o.ap()[:, GW * g + GW // 2 * h :
                                            GW * g + GW // 2 * (h + 1)],
                                gsl[:, hs],
                            )
                    else:
                        nc.vector.tensor_add(out=gsl, in0=gsl, in1=e)
                # racc[g] complete: ship it out now, overlapping next g
                if g != G - 1:
                    nc.sync.dma_start(racc_o.ap()[:, GW * g : GW * (g + 1)], gsl)

        nc.vector.reduce_sum(rows, rowacc[:, :, :], axis=mybir.AxisListType.X)
        nc.sync.dma_start(rowsum_o.ap(), rows)

    nc.compile()
    return nc


def _ensure_ntff_hook():
    """antenv.axon_hooks is absent on this image; provide the tiny get/set
    registry and register trn_agent_boot's ctypes NTFF hook so trace=True
    works. Only used from test runs (KERNEL_TRACE=1)."""
    import sys
    import types

    try:
        import antenv.axon_hooks  # noqa: F401
        return
    except ImportError:
        pass
    mod = types.ModuleType("antenv.axon_hooks")
    _state = {"hook": None}
    mod.set_axon_ntff_profile_hook = lambda h: _state.__setitem__("hook", h)
    mod.get_axon_ntff_profile_hook = lambda: _state["hook"]
    import antenv

    sys.modules["antenv.axon_hooks"] = mod
    antenv.axon_hooks = mod
    try:
        from trn_agent_boot.trn_boot import _ntff_profile_via_ctypes

        mod.set_axon_ntff_profile_hook(
            _ntff_profile_via_ctypes("/opt/axon/libaxon_pjrt.so")
        )
    except Exception as e:  # degrade to no tracing
        print(f"NTFF hook setup failed: {e}")


def kernel(image_features, spectrum_features, logit_scale):
    scale = float(np.asarray(logit_scale))
    key = round(scale, 9)
    if key not in _cache:
        _cache[key] = _build(scale)
    nc = _cache[key]

    import ml_dtypes

    img = np.asarray(image_features, dtype=np.float32)
    spec = np.asarray(spectrum_features, dtype=np.float32)
    imgN = img / np.maximum(
        np.sqrt((img * img).sum(axis=1, keepdims=True)), 1e-3
    )
    specN = spec / np.maximum(
        np.sqrt((spec * spec).sum(axis=1, keepdims=True)), 1e-3
    )
    diag_sum = scale * float(
        np.einsum("nd,nd->", imgN.astype(np.float64), specN.astype(np.float64))
    )

    f8 = ml_dtypes.float8_e4m3fn
    # [p, k, n] = xN[n, 128k + p] * 16 — the PE lhsT/rhs chunk-major layout
    specT8 = np.ascontiguousarray(
        (specN.T * FP8_PRESCALE).astype(f8).reshape(KC, P, N).transpose(1, 0, 2)
    )
    imgT8_all = (imgN.T * FP8_PRESCALE).astype(f8)  # [D, N]
    in_maps = []
    for c in range(C):
        imgT8 = np.ascontiguousarray(
            imgT8_all[:, c * NL : (c + 1) * NL].reshape(KC, P, NL).transpose(1, 0, 2)
        )
        in_maps.append({"imgT": imgT8, "specT": specT8})

    trace = os.environ.get("KERNEL_TRACE") == "1"
    if trace:
        _ensure_ntff_hook()
    res = run_bass_kernel_spmd(nc, in_maps, core_ids=list(range(C)), trace=trace)
    if trace:
        print(f"HW exec time: {res.exec_time_ns} ns (mean {res.mean_exec_time_ns})")

    rs = np.stack([r["rowsum"] for r in res.results]).astype(np.float64)  # [C,P,T]
    cs = np.stack(
        [r["racc_o"].astype(np.float64).sum(axis=0) for r in res.results]
    )  # [C,N]

    lse_i_sum = float(np.sum(np.log(rs)))
    lse_s_sum = float(np.sum(np.log(cs.sum(axis=0))))
    loss = 0.5 * ((lse_i_sum - diag_sum) / N + (lse_s_sum - diag_sum) / N)
    return np.float32(loss)


# revision 8
# speedup vs baseline: 1.1542x; 1.1542x over previous
"""CLIP loss kernel for trn2, 8 NeuronCores, data-parallel over the batch dim.

Strategy (per core c of 8, SPMD) — no collectives:
  The host pre-normalizes both modalities (x / max(|x|, 1e-3), matching the
  reference), pre-transposes them into PE lhsT/rhs layout, scales by 16 and
  casts to fp8e4m3 (entries of unit rows are <= 1, so x16 uses fp8's range).
  Each core receives its own 1024 img rows (transposed, [128, 4, 1024]) plus
  the FULL transposed spec matrix ([128, 4, 8192]) — replicating 4 MB of fp8
  to every core replaces the AllGather + mesh barrier of the collective
  formulation, which otherwise serializes ~60us at the head of the kernel.

  Device work per core is a single pipeline:
    logits block [1024, 8192] = imgT.T @ specT, fp8 DoubleRow matmuls
    (K=256 per pass, PSUM f32, [128, 2048] tiles), then ACT Exp with
    scale = logit_scale/256 (the 16x16 fp8 prescale cancels); accum_out
    yields row-sums of exp for free; the exp tile (bf16, SBUF) accumulates
    into racc[128, 8192] (DVE add) = column partial sums stratified by
    partition.
  Host: log/sum of row sums and column sums (O(N) numpy), diagonal term
  computed directly on the host in f64 -> scalar loss.
"""

import os
from contextlib import ExitStack

import numpy as np

import concourse.bass as bass
import concourse.mybir as mybir
from concourse import bacc, tile
from concourse.bass_utils import run_bass_kernel_spmd

N, D, C = 8192, 512, 8
NL = N // C  # 1024 local rows per core
P = 128
T = NL // P  # 8 [128 row] tiles per core
KC = D // P  # 4 contraction chunks of 128
G = 4        # column groups
GW = N // G  # 2048 columns per group

f32 = mybir.dt.float32
bf16 = mybir.dt.bfloat16
fp8 = mybir.dt.float8e4
FA = mybir.ActivationFunctionType

# operands are pre-scaled by 16 on the host to center fp8's dynamic range;
# the matmul result is 256x too big, compensated in the exp scale.
FP8_PRESCALE = 16.0

_cache: dict = {}


def _build(scale: float):
    nc = bacc.Bacc("TRN2", target_bir_lowering=False, debug=False, num_devices=C)
    imgT_d = nc.dram_tensor("imgT", [P, KC, NL], fp8, kind="ExternalInput")
    specT_d = nc.dram_tensor("specT", [P, KC, N], fp8, kind="ExternalInput")
    rowsum_o = nc.dram_tensor("rowsum", [P, T], f32, kind="ExternalOutput")
    racc_o = nc.dram_tensor("racc_o", [P, N], bf16, kind="ExternalOutput")

    with tile.TileContext(nc) as tc, ExitStack() as ctx:
        const = ctx.enter_context(tc.tile_pool(name="const", bufs=1))
        pers = ctx.enter_context(tc.tile_pool(name="pers", bufs=1))
        ps = ctx.enter_context(tc.tile_pool(name="ps", bufs=2, space="PSUM"))
        ep = ctx.enter_context(tc.tile_pool(name="e", bufs=4))

        imgT = pers.tile([P, KC, NL], fp8, name="imgT")
        specT = pers.tile([P, KC, N], fp8, name="specT")
        racc = pers.tile([P, N], bf16, name="racc")
        rowacc = pers.tile([P, T, G], f32, name="rowacc")
        rows = pers.tile([P, T], f32, name="rows")

        # preload the exp activation table while the input DMAs run
        warm = const.tile([P, 1], f32, name="actwarm")
        nc.vector.memset(warm, 1.0)
        nc.scalar.activation(warm, warm, FA.Exp)

        # input DMAs, ordered so the (g=0, m=0) matmuls can start earliest:
        # img lhsT m=0 slice, the whole of spec group 0, the rest of img,
        # then groups 1-3.
        nc.sync.dma_start(imgT[:, :, 0:P], imgT_d.ap()[:, :, 0:P])
        nc.sync.dma_start(specT[:, :, 0:GW], specT_d.ap()[:, :, 0:GW])
        nc.sync.dma_start(imgT[:, :, P:NL], imgT_d.ap()[:, :, P:NL])
        for g in range(1, G):
            nc.sync.dma_start(
                specT[:, :, GW * g : GW * (g + 1)],
                specT_d.ap()[:, :, GW * g : GW * (g + 1)],
            )

        # main loop: logits block, exp, row/col accumulation
        with nc.allow_low_precision("bf16 exp-sum accumulation, <1e-3 on loss"):
            for g in range(G):
                gsl = racc[:, GW * g : GW * (g + 1)]
                for m in range(T):
                    pm = ps.tile([P, GW], f32, tag="mm")
                    # fp8 DoubleRow: each matmul contracts 2 k-chunks (K=256)
                    for q in range(KC // 2):
                        for ns in range(GW // 512):
                            cs = slice(GW * g + 512 * ns, GW * g + 512 * (ns + 1))
                            nc.tensor.matmul(
                                pm[:, 512 * ns : 512 * (ns + 1)],
                                imgT[:, 2 * q : 2 * q + 2, P * m : P * (m + 1)],
                                specT[:, 2 * q : 2 * q + 2, cs],
                                start=(q == 0),
                                stop=(q == KC // 2 - 1),
                                perf_mode=mybir.MatmulPerfMode.DoubleRow,
                            )
                    e = ep.tile([P, GW], bf16, tag="e")
                    nc.scalar.activation(
                        e, pm, FA.Exp,
                        scale=scale / (FP8_PRESCALE * FP8_PRESCALE),
                        accum_out=rowacc[:, m, g : g + 1],
                    )
                    if m == 0:
                        nc.vector.tensor_copy(gsl, e)
                    elif m == T - 1 and g == G - 1:
                        # final tile: halve the add so the write-out overlaps
                        for h in range(2):
                            hs = slice(GW // 2 * h, GW // 2 * (h + 1))
                            nc.vector.tensor_add(
                                out=gsl[:, hs], in0=gsl[:, hs], in1=e[:, hs]
                            )
                            nc.sync.dma_start(
                                racc_o.ap()[:, GW * g + GW // 2 * h :
                                            GW * g + GW // 2 * (h + 1)],
                                gsl[:, hs],
                            )
                    else:
                        nc.vector.tensor_add(out=gsl, in0=gsl, in1=e)
                # racc[g] complete: ship it out now, overlapping next g
                if g != G - 1:
                    nc.sync.dma_start(racc_o.ap()[:, GW * g : GW * (g + 1)], gsl)

        nc.vector.reduce_sum(rows, rowacc[:, :, :], axis=mybir.AxisListType.X)
        nc.sync.dma_start(rowsum_o.ap(), rows)

    nc.compile()
    return nc


def _ensure_ntff_hook():
    """antenv.axon_hooks is absent on this image; provide the tiny get/set
    registry and register trn_agent_boot's ctypes NTFF hook so trace=True
    works. Only used from test runs (KERNEL_TRACE=1)."""
    import sys
    import types

    try:
        import antenv.axon_hooks  # noqa: F401
        return
    except ImportError:
        pass
    mod = types.ModuleType("antenv.axon_hooks")
    _state = {"hook": None}
    mod.set_axon_ntff_profile_hook = lambda h: _state.__setitem__("hook", h)
    mod.get_axon_ntff_profile_hook = lambda: _state["hook"]
    import antenv

    sys.modules["antenv.axon_hooks"] = mod
    antenv.axon_hooks = mod
    try:
        from trn_agent_boot.trn_boot import _ntff_profile_via_ctypes

        mod.set_axon_ntff_profile_hook(
            _ntff_profile_via_ctypes("/opt/axon/libaxon_pjrt.so")
        )
    except Exception as e:  # degrade to no tracing
        print(f"NTFF hook setup failed: {e}")


def kernel(image_features, spectrum_features, logit_scale):
    scale = float(np.asarray(logit_scale))
    key = round(scale, 9)
    if key not in _cache:
        _cache[key] = _build(scale)
    nc = _cache[key]

    import ml_dtypes

    img = np.asarray(image_features, dtype=np.float32)
    spec = np.asarray(spectrum_features, dtype=np.float32)
    imgN = img / np.maximum(
        np.sqrt((img * img).sum(axis=1, keepdims=True)), 1e-3
    )
    specN = spec / np.maximum(
        np.sqrt((spec * spec).sum(axis=1, keepdims=True)), 1e-3
    )
    diag_sum = scale * float(
        np.einsum("nd,nd->", imgN.astype(np.float64), specN.astype(np.float64))
    )

    f8 = ml_dtypes.float8_e4m3fn
    # [p, k, n] = xN[n, 128k + p] * 16 — the PE lhsT/rhs chunk-major layout
    specT8 = np.ascontiguousarray(
        (specN.T * FP8_PRESCALE).astype(f8).reshape(KC, P, N).transpose(1, 0, 2)
    )
    imgT8_all = (imgN.T * FP8_PRESCALE).astype(f8)  # [D, N]
    in_maps = []
    for c in range(C):
        imgT8 = np.ascontiguousarray(
            imgT8_all[:, c * NL : (c + 1) * NL].reshape(KC, P, NL).transpose(1, 0, 2)
        )
        in_maps.append({"imgT": imgT8, "specT": specT8})

    trace = os.environ.get("KERNEL_TRACE") == "1"
    if trace:
        _ensure_ntff_hook()
    res = run_bass_kernel_spmd(nc, in_maps, core_ids=list(range(C)), trace=trace)
    if trace:
        print(f"HW exec time: {res.exec_time_ns} ns (mean {res.mean_exec_time_ns})")

    rs = np.stack([r["rowsum"] for r in res.results]).astype(np.float64)  # [C,P,T]
    cs = np.stack(
        [r["racc_o"].astype(np.float64).sum(axis=0) for r in res.results]
    )  # [C,N]

    lse_i_sum = float(np.sum(np.log(rs)))
    lse_s_sum = float(np.sum(np.log(cs.sum(axis=0))))
    loss = 0.5 * ((lse_i_sum - diag_sum) / N + (lse_s_sum - diag_sum) / N)
    return np.float32(loss)


# revision 14
# speedup vs baseline: 1.1861x; 1.0277x over previous
"""CLIP loss kernel for trn2, 8 NeuronCores, data-parallel over the batch dim.

Strategy (per core c of 8, SPMD) — no collectives:
  The host pre-normalizes both modalities (x / max(|x|, 1e-3), matching the
  reference), pre-transposes them into PE lhsT/rhs layout, scales by 16 and
  casts to fp8e4m3 (entries of unit rows are <= 1, so x16 uses fp8's range).
  Each core receives its own 1024 img rows (transposed, [128, 4, 1024]) plus
  the FULL transposed spec matrix ([128, 4, 8192]) — replicating 4 MB of fp8
  to every core replaces the AllGather + mesh barrier of the collective
  formulation, which otherwise serializes ~60us at the head of the kernel.

  Device work per core is a single pipeline:
    logits block [1024, 8192] = imgT.T @ specT, fp8 DoubleRow matmuls
    (K=256 per pass, PSUM f32, [128, 2048] tiles), then ACT Exp with
    scale = logit_scale/256 (the 16x16 fp8 prescale cancels); accum_out
    yields row-sums of exp for free; the exp tile (bf16, SBUF) accumulates
    into racc[128, 8192] (DVE add) = column partial sums stratified by
    partition.
  Host: log/sum of row sums and column sums (O(N) numpy), diagonal term
  computed directly on the host in f64 -> scalar loss.
"""

import os
from contextlib import ExitStack

import numpy as np

import concourse.bass as bass
import concourse.mybir as mybir
from concourse import bacc, tile
from concourse.bass_utils import run_bass_kernel_spmd

N, D, C = 8192, 512, 8
NL = N // C  # 1024 local rows per core
P = 128
T = NL // P  # 8 [128 row] tiles per core
KC = D // P  # 4 contraction chunks of 128
G = 4        # column groups
GW = N // G  # 2048 columns per group

f32 = mybir.dt.float32
bf16 = mybir.dt.bfloat16
fp8 = mybir.dt.float8e4
FA = mybir.ActivationFunctionType

# operands are pre-scaled by 16 on the host to center fp8's dynamic range;
# the matmul result is 256x too big, compensated in the exp scale.
FP8_PRESCALE = 16.0

_cache: dict = {}


def _build(scale: float):
    nc = bacc.Bacc("TRN2", target_bir_lowering=False, debug=False, num_devices=C)
    # img lhsT is m-major so each m-tile's weights are one contiguous run
    imgT_d = nc.dram_tensor("imgT", [P, T, KC, P], fp8, kind="ExternalInput")
    specT_d = nc.dram_tensor("specT", [P, KC, N], fp8, kind="ExternalInput")
    rowsum_o = nc.dram_tensor("rowsum", [P, T], f32, kind="ExternalOutput")
    racc_o = nc.dram_tensor("racc_o", [P, N], bf16, kind="ExternalOutput")

    with tile.TileContext(nc) as tc, ExitStack() as ctx:
        const = ctx.enter_context(tc.tile_pool(name="const", bufs=1))
        pers = ctx.enter_context(tc.tile_pool(name="pers", bufs=1))
        ps = ctx.enter_context(tc.tile_pool(name="ps", bufs=2, space="PSUM"))
        ep = ctx.enter_context(tc.tile_pool(name="e", bufs=4))

        imgT = pers.tile([P, T, KC, P], fp8, name="imgT")
        specT = pers.tile([P, KC, N], fp8, name="specT")
        racc = pers.tile([P, N], bf16, name="racc")
        rowacc = pers.tile([P, T, G], f32, name="rowacc")
        rows = pers.tile([P, T], f32, name="rows")

        # preload the exp activation table while the input DMAs run
        warm = const.tile([P, 1], f32, name="actwarm")
        nc.vector.memset(warm, 1.0)
        nc.scalar.activation(warm, warm, FA.Exp)

        # input DMAs, ordered so the (g=0, m=0) matmuls can start earliest:
        # img lhsT m=0 slice, group-0 spec k-chunks 0-1 (all the q=0 matmuls
        # need), chunks 2-3, the rest of img, then groups 1-3 whole.
        nc.sync.dma_start(imgT[:, 0], imgT_d.ap()[:, 0])
        nc.sync.dma_start(specT[:, 0:2, 0:GW], specT_d.ap()[:, 0:2, 0:GW])
        nc.sync.dma_start(specT[:, 2:4, 0:GW], specT_d.ap()[:, 2:4, 0:GW])
        nc.sync.dma_start(imgT[:, 1:T], imgT_d.ap()[:, 1:T])
        for g in range(1, G):
            nc.sync.dma_start(
                specT[:, :, GW * g : GW * (g + 1)],
                specT_d.ap()[:, :, GW * g : GW * (g + 1)],
            )

        # ramp the PE p-state while the input DMAs stream: dummy matmuls on
        # a memset tile keep the tensor clock up so the first real matmuls
        # don't pay the slow-start penalty
        wsrc = const.tile([P, 64], fp8, name="pewarm")
        nc.vector.memset(wsrc, 0.0)
        for w in range(18):
            wp = ps.tile([P, GW], f32, tag="mm")
            nc.tensor.matmul(
                wp[0:64, 0:64], wsrc, wsrc, start=True, stop=True
            )

        # main loop: logits block, exp, row/col accumulation
        with nc.allow_low_precision("bf16 exp-sum accumulation, <1e-3 on loss"):
            for g in range(G):
                gsl = racc[:, GW * g : GW * (g + 1)]
                for m in range(T):
                    pm = ps.tile([P, GW], f32, tag="mm")
                    # fp8 DoubleRow: each matmul contracts 2 k-chunks (K=256)
                    for q in range(KC // 2):
                        for ns in range(GW // 512):
                            cs = slice(GW * g + 512 * ns, GW * g + 512 * (ns + 1))
                            nc.tensor.matmul(
                                pm[:, 512 * ns : 512 * (ns + 1)],
                                imgT[:, m, 2 * q : 2 * q + 2, :],
                                specT[:, 2 * q : 2 * q + 2, cs],
                                start=(q == 0),
                                stop=(q == KC // 2 - 1),
                                perf_mode=mybir.MatmulPerfMode.DoubleRow,
                            )
                    e = ep.tile([P, GW], bf16, tag="e")
                    nc.scalar.activation(
                        e, pm, FA.Exp,
                        scale=scale / (FP8_PRESCALE * FP8_PRESCALE),
                        accum_out=rowacc[:, m, g : g + 1],
                    )
                    if m == 0:
                        nc.vector.tensor_copy(gsl, e)
                    elif m == T - 1 and g == G - 1:
                        # final tile: halve the add so the write-out overlaps
                        for h in range(2):
                            hs = slice(GW // 2 * h, GW // 2 * (h + 1))
                            nc.vector.tensor_add(
                                out=gsl[:, hs], in0=gsl[:, hs], in1=e[:, hs]
                            )
                            nc.sync.dma_start(
                                racc_o.ap()[:, GW * g + GW // 2 * h :
                                            GW * g + GW // 2 * (h + 1)],
                                gsl[:, hs],
                            )
                    else:
                        nc.vector.tensor_add(out=gsl, in0=gsl, in1=e)
                # racc[g] complete: ship it out now, overlapping next g
                if g != G - 1:
                    nc.sync.dma_start(racc_o.ap()[:, GW * g : GW * (g + 1)], gsl)

        nc.vector.reduce_sum(rows, rowacc[:, :, :], axis=mybir.AxisListType.X)
        nc.sync.dma_start(rowsum_o.ap(), rows)

    nc.compile()
    return nc


def _ensure_ntff_hook():
    """antenv.axon_hooks is absent on this image; provide the tiny get/set
    registry and register trn_agent_boot's ctypes NTFF hook so trace=True
    works. Only used from test runs (KERNEL_TRACE=1)."""
    import sys
    import types

    try:
        import antenv.axon_hooks  # noqa: F401
        return
    except ImportError:
        pass
    mod = types.ModuleType("antenv.axon_hooks")
    _state = {"hook": None}
    mod.set_axon_ntff_profile_hook = lambda h: _state.__setitem__("hook", h)
    mod.get_axon_ntff_profile_hook = lambda: _state["hook"]
    import antenv

    sys.modules["antenv.axon_hooks"] = mod
    antenv.axon_hooks = mod
    try:
        from trn_agent_boot.trn_boot import _ntff_profile_via_ctypes

        mod.set_axon_ntff_profile_hook(
            _ntff_profile_via_ctypes("/opt/axon/libaxon_pjrt.so")
        )
    except Exception as e:  # degrade to no tracing
        print(f"NTFF hook setup failed: {e}")


def kernel(image_features, spectrum_features, logit_scale):
    scale = float(np.asarray(logit_scale))
    key = round(scale, 9)
    if key not in _cache:
        _cache[key] = _build(scale)
    nc = _cache[key]

    import ml_dtypes

    img = np.asarray(image_features, dtype=np.float32)
    spec = np.asarray(spectrum_features, dtype=np.float32)
    imgN = img / np.maximum(
        np.sqrt((img * img).sum(axis=1, keepdims=True)), 1e-3
    )
    specN = spec / np.maximum(
        np.sqrt((spec * spec).sum(axis=1, keepdims=True)), 1e-3
    )
    diag_sum = scale * float(
        np.einsum("nd,nd->", imgN.astype(np.float64), specN.astype(np.float64))
    )

    f8 = ml_dtypes.float8_e4m3fn
    # [p, k, n] = xN[n, 128k + p] * 16 — the PE lhsT/rhs chunk-major layout
    specT8 = np.ascontiguousarray(
        (specN.T * FP8_PRESCALE).astype(f8).reshape(KC, P, N).transpose(1, 0, 2)
    )
    imgT8_all = (imgN.T * FP8_PRESCALE).astype(f8)  # [D, N]
    in_maps = []
    for c in range(C):
        # [p, m, k, j] = imgN[c*NL + 128m + j, 128k + p] * 16
        imgT8 = np.ascontiguousarray(
            imgT8_all[:, c * NL : (c + 1) * NL]
            .reshape(KC, P, T, P)
            .transpose(1, 2, 0, 3)
        )
        in_maps.append({"imgT": imgT8, "specT": specT8})

    trace = os.environ.get("KERNEL_TRACE") == "1"
    if trace:
        _ensure_ntff_hook()
    res = run_bass_kernel_spmd(nc, in_maps, core_ids=list(range(C)), trace=trace)
    if trace:
        print(f"HW exec time: {res.exec_time_ns} ns (mean {res.mean_exec_time_ns})")

    rs = np.stack([r["rowsum"] for r in res.results]).astype(np.float64)  # [C,P,T]
    cs = np.stack(
        [r["racc_o"].astype(np.float64).sum(axis=0) for r in res.results]
    )  # [C,N]

    lse_i_sum = float(np.sum(np.log(rs)))
    lse_s_sum = float(np.sum(np.log(cs.sum(axis=0))))
    loss = 0.5 * ((lse_i_sum - diag_sum) / N + (lse_s_sum - diag_sum) / N)
    return np.float32(loss)


# revision 18
# speedup vs baseline: 1.2033x; 1.0145x over previous
"""CLIP loss kernel for trn2, 8 NeuronCores, data-parallel over the batch dim.

Strategy (per core c of 8, SPMD) — no collectives:
  The host pre-normalizes both modalities (x / max(|x|, 1e-3), matching the
  reference), pre-transposes them into PE lhsT/rhs layout, scales by 16 and
  casts to fp8e4m3 (entries of unit rows are <= 1, so x16 uses fp8's range).
  Each core receives its own 1024 img rows (transposed, [128, 4, 1024]) plus
  the FULL transposed spec matrix ([128, 4, 8192]) — replicating 4 MB of fp8
  to every core replaces the AllGather + mesh barrier of the collective
  formulation, which otherwise serializes ~60us at the head of the kernel.

  Device work per core is a single pipeline:
    logits block [1024, 8192] = imgT.T @ specT, fp8 DoubleRow matmuls
    (K=256 per pass, PSUM f32, [128, 2048] tiles), then ACT Exp with
    scale = logit_scale/256 (the 16x16 fp8 prescale cancels); accum_out
    yields row-sums of exp for free; the exp tile (bf16, SBUF) accumulates
    into racc[128, 8192] (DVE add) = column partial sums stratified by
    partition.
  Host: log/sum of row sums and column sums (O(N) numpy), diagonal term
  computed directly on the host in f64 -> scalar loss.
"""

import os
from contextlib import ExitStack

import numpy as np

import concourse.bass as bass
import concourse.mybir as mybir
from concourse import bacc, tile
from concourse.bass_utils import run_bass_kernel_spmd

N, D, C = 8192, 512, 8
NL = N // C  # 1024 local rows per core
P = 128
T = NL // P  # 8 [128 row] tiles per core
KC = D // P  # 4 contraction chunks of 128
G = 4        # column groups
GW = N // G  # 2048 columns per group

f32 = mybir.dt.float32
bf16 = mybir.dt.bfloat16
fp8 = mybir.dt.float8e4
FA = mybir.ActivationFunctionType

# operands are pre-scaled by 16 on the host to center fp8's dynamic range;
# the matmul result is 256x too big, compensated in the exp scale.
FP8_PRESCALE = 16.0

_cache: dict = {}


def _build(scale: float):
    nc = bacc.Bacc("TRN2", target_bir_lowering=False, debug=False, num_devices=C)
    # img lhsT is m-major so each m-tile's weights are one contiguous run
    imgT_d = nc.dram_tensor("imgT", [P, T, KC, P], fp8, kind="ExternalInput")
    specT_d = nc.dram_tensor("specT", [P, KC, N], fp8, kind="ExternalInput")
    rowsum_o = nc.dram_tensor("rowsum", [P, T], f32, kind="ExternalOutput")
    racc_o = nc.dram_tensor("racc_o", [P, N], bf16, kind="ExternalOutput")

    with tile.TileContext(nc) as tc, ExitStack() as ctx:
        pers = ctx.enter_context(tc.tile_pool(name="pers", bufs=1))
        ps = ctx.enter_context(tc.tile_pool(name="ps", bufs=2, space="PSUM"))

        imgT = pers.tile([P, T, KC, P], fp8, name="imgT")
        specT = pers.tile([P, KC, N], fp8, name="specT")
        racc = pers.tile([P, N], bf16, name="racc")
        rowacc = pers.tile([P, T, G], f32, name="rowacc")
        rows = pers.tile([P, T], f32, name="rows")
        # manually-rotated exp tiles (a named ring instead of a pool buys one
        # fewer pool-close drain round at kernel end)
        e_tiles = [pers.tile([P, GW], bf16, name=f"e{i}") for i in range(4)]

        # preload the exp activation table while the input DMAs run
        warm = pers.tile([P, 1], f32, name="actwarm")
        nc.vector.memset(warm, 1.0)
        nc.scalar.activation(warm, warm, FA.Exp)

        # input DMAs, ordered so the (g=0, m=0) matmuls can start earliest:
        # img lhsT m=0 slice, group-0 spec k-chunks 0-1 (all the q=0 matmuls
        # need), chunks 2-3, the rest of img, then groups 1-3 whole.
        nc.sync.dma_start(imgT[:, 0], imgT_d.ap()[:, 0])
        nc.sync.dma_start(specT[:, 0:2, 0:GW], specT_d.ap()[:, 0:2, 0:GW])
        nc.sync.dma_start(specT[:, 2:4, 0:GW], specT_d.ap()[:, 2:4, 0:GW])
        nc.sync.dma_start(imgT[:, 1:T], imgT_d.ap()[:, 1:T])
        for g in range(1, G):
            nc.sync.dma_start(
                specT[:, :, GW * g : GW * (g + 1)],
                specT_d.ap()[:, :, GW * g : GW * (g + 1)],
            )

        # ramp the PE p-state while the input DMAs stream: dummy matmuls on
        # a memset tile keep the tensor clock up so the first real matmuls
        # don't pay the slow-start penalty
        wsrc = pers.tile([P, 64], fp8, name="pewarm")
        nc.vector.memset(wsrc, 0.0)
        for w in range(26):
            wp = ps.tile([P, GW], f32, tag="mm")
            nc.tensor.matmul(
                wp[0:64, 0:64], wsrc, wsrc, start=True, stop=True
            )

        # main loop: logits block, exp, row/col accumulation
        with nc.allow_low_precision("bf16 exp-sum accumulation, <1e-3 on loss"):
            for g in range(G):
                gsl = racc[:, GW * g : GW * (g + 1)]
                for m in range(T):
                    pm = ps.tile([P, GW], f32, tag="mm")
                    # fp8 DoubleRow: each matmul contracts 2 k-chunks (K=256)
                    for q in range(KC // 2):
                        for ns in range(GW // 512):
                            cs = slice(GW * g + 512 * ns, GW * g + 512 * (ns + 1))
                            nc.tensor.matmul(
                                pm[:, 512 * ns : 512 * (ns + 1)],
                                imgT[:, m, 2 * q : 2 * q + 2, :],
                                specT[:, 2 * q : 2 * q + 2, cs],
                                start=(q == 0),
                                stop=(q == KC // 2 - 1),
                                perf_mode=mybir.MatmulPerfMode.DoubleRow,
                            )
                    e = e_tiles[(g * T + m) % 4]
                    nc.scalar.activation(
                        e, pm, FA.Exp,
                        scale=scale / (FP8_PRESCALE * FP8_PRESCALE),
                        accum_out=rowacc[:, m, g : g + 1],
                    )
                    if m == 0:
                        nc.vector.tensor_copy(gsl, e)
                    elif m == T - 1 and g == G - 1:
                        # final tile: halve the add so the write-out overlaps
                        for h in range(2):
                            hs = slice(GW // 2 * h, GW // 2 * (h + 1))
                            nc.vector.tensor_add(
                                out=gsl[:, hs], in0=gsl[:, hs], in1=e[:, hs]
                            )
                            nc.sync.dma_start(
                                racc_o.ap()[:, GW * g + GW // 2 * h :
                                            GW * g + GW // 2 * (h + 1)],
                                gsl[:, hs],
                            )
                    else:
                        nc.vector.tensor_add(out=gsl, in0=gsl, in1=e)
                # racc[g] complete: ship it out now, overlapping next g
                if g != G - 1:
                    nc.sync.dma_start(racc_o.ap()[:, GW * g : GW * (g + 1)], gsl)

        nc.vector.reduce_sum(rows, rowacc[:, :, :], axis=mybir.AxisListType.X)
        nc.sync.dma_start(rowsum_o.ap(), rows)

    nc.compile()
    return nc


def _ensure_ntff_hook():
    """antenv.axon_hooks is absent on this image; provide the tiny get/set
    registry and register trn_agent_boot's ctypes NTFF hook so trace=True
    works. Only used from test runs (KERNEL_TRACE=1)."""
    import sys
    import types

    try:
        import antenv.axon_hooks  # noqa: F401
        return
    except ImportError:
        pass
    mod = types.ModuleType("antenv.axon_hooks")
    _state = {"hook": None}
    mod.set_axon_ntff_profile_hook = lambda h: _state.__setitem__("hook", h)
    mod.get_axon_ntff_profile_hook = lambda: _state["hook"]
    import antenv

    sys.modules["antenv.axon_hooks"] = mod
    antenv.axon_hooks = mod
    try:
        from trn_agent_boot.trn_boot import _ntff_profile_via_ctypes

        mod.set_axon_ntff_profile_hook(
            _ntff_profile_via_ctypes("/opt/axon/libaxon_pjrt.so")
        )
    except Exception as e:  # degrade to no tracing
        print(f"NTFF hook setup failed: {e}")


def kernel(image_features, spectrum_features, logit_scale):
    scale = float(np.asarray(logit_scale))
    key = round(scale, 9)
    if key not in _cache:
        _cache[key] = _build(scale)
    nc = _cache[key]

    import ml_dtypes

    img = np.asarray(image_features, dtype=np.float32)
    spec = np.asarray(spectrum_features, dtype=np.float32)
    imgN = img / np.maximum(
        np.sqrt((img * img).sum(axis=1, keepdims=True)), 1e-3
    )
    specN = spec / np.maximum(
        np.sqrt((spec * spec).sum(axis=1, keepdims=True)), 1e-3
    )
    diag_sum = scale * float(
        np.einsum("nd,nd->", imgN.astype(np.float64), specN.astype(np.float64))
    )

    f8 = ml_dtypes.float8_e4m3fn
    # [p, k, n] = xN[n, 128k + p] * 16 — the PE lhsT/rhs chunk-major layout
    specT8 = np.ascontiguousarray(
        (specN.T * FP8_PRESCALE).astype(f8).reshape(KC, P, N).transpose(1, 0, 2)
    )
    imgT8_all = (imgN.T * FP8_PRESCALE).astype(f8)  # [D, N]
    in_maps = []
    for c in range(C):
        # [p, m, k, j] = imgN[c*NL + 128m + j, 128k + p] * 16
        imgT8 = np.ascontiguousarray(
            imgT8_all[:, c * NL : (c + 1) * NL]
            .reshape(KC, P, T, P)
            .transpose(1, 2, 0, 3)
        )
        in_maps.append({"imgT": imgT8, "specT": specT8})

    trace = os.environ.get("KERNEL_TRACE") == "1"
    if trace:
        _ensure_ntff_hook()
    res = run_bass_kernel_spmd(nc, in_maps, core_ids=list(range(C)), trace=trace)
    if trace:
        print(f"HW exec time: {res.exec_time_ns} ns (mean {res.mean_exec_time_ns})")

    rs = np.stack([r["rowsum"] for r in res.results]).astype(np.float64)  # [C,P,T]
    cs = np.stack(
        [r["racc_o"].astype(np.float64).sum(axis=0) for r in res.results]
    )  # [C,N]

    lse_i_sum = float(np.sum(np.log(rs)))
    lse_s_sum = float(np.sum(np.log(cs.sum(axis=0))))
    loss = 0.5 * ((lse_i_sum - diag_sum) / N + (lse_s_sum - diag_sum) / N)
    return np.float32(loss)


# revision 19
# speedup vs baseline: 1.2061x; 1.0023x over previous
"""CLIP loss kernel for trn2, 8 NeuronCores, data-parallel over the batch dim.

Strategy (per core c of 8, SPMD) — no collectives:
  The host pre-normalizes both modalities (x / max(|x|, 1e-3), matching the
  reference), pre-transposes them into PE lhsT/rhs layout, scales by 16 and
  casts to fp8e4m3 (entries of unit rows are <= 1, so x16 uses fp8's range).
  Each core receives its own 1024 img rows (transposed, [128, 4, 1024]) plus
  the FULL transposed spec matrix ([128, 4, 8192]) — replicating 4 MB of fp8
  to every core replaces the AllGather + mesh barrier of the collective
  formulation, which otherwise serializes ~60us at the head of the kernel.

  Device work per core is a single pipeline:
    logits block [1024, 8192] = imgT.T @ specT, fp8 DoubleRow matmuls
    (K=256 per pass, PSUM f32, [128, 2048] tiles), then ACT Exp with
    scale = logit_scale/256 (the 16x16 fp8 prescale cancels); accum_out
    yields row-sums of exp for free; the exp tile (bf16, SBUF) accumulates
    into racc[128, 8192] (DVE add) = column partial sums stratified by
    partition.
  Host: log/sum of row sums and column sums (O(N) numpy), diagonal term
  computed directly on the host in f64 -> scalar loss.
"""

import os
from contextlib import ExitStack

import numpy as np

import concourse.bass as bass
import concourse.mybir as mybir
from concourse import bacc, tile
from concourse.bass_utils import run_bass_kernel_spmd

N, D, C = 8192, 512, 8
NL = N // C  # 1024 local rows per core
P = 128
T = NL // P  # 8 [128 row] tiles per core
KC = D // P  # 4 contraction chunks of 128
G = 4        # column groups
GW = N // G  # 2048 columns per group

f32 = mybir.dt.float32
bf16 = mybir.dt.bfloat16
fp8 = mybir.dt.float8e4
FA = mybir.ActivationFunctionType

# operands are pre-scaled by 16 on the host to center fp8's dynamic range;
# the matmul result is 256x too big, compensated in the exp scale.
FP8_PRESCALE = 16.0

_cache: dict = {}


def _build(scale: float):
    nc = bacc.Bacc("TRN2", target_bir_lowering=False, debug=False, num_devices=C)
    # img lhsT is m-major so each m-tile's weights are one contiguous run
    imgT_d = nc.dram_tensor("imgT", [P, T, KC, P], fp8, kind="ExternalInput")
    specT_d = nc.dram_tensor("specT", [P, KC, N], fp8, kind="ExternalInput")
    rowsum_o = nc.dram_tensor("rowsum", [P, T], f32, kind="ExternalOutput")
    racc_o = nc.dram_tensor("racc_o", [P, N], bf16, kind="ExternalOutput")

    with tile.TileContext(nc) as tc, ExitStack() as ctx:
        pers = ctx.enter_context(tc.tile_pool(name="pers", bufs=1))
        ps = ctx.enter_context(tc.tile_pool(name="ps", bufs=2, space="PSUM"))

        imgT = pers.tile([P, T, KC, P], fp8, name="imgT")
        specT = pers.tile([P, KC, N], fp8, name="specT")
        racc = pers.tile([P, N], bf16, name="racc")
        rowacc = pers.tile([P, T, G], f32, name="rowacc")
        rows = pers.tile([P, T], f32, name="rows")
        # manually-rotated exp tiles (a named ring instead of a pool buys one
        # fewer pool-close drain round at kernel end)
        e_tiles = [pers.tile([P, GW], bf16, name=f"e{i}") for i in range(4)]

        # preload the exp activation table while the input DMAs run
        warm = pers.tile([P, 1], f32, name="actwarm")
        nc.vector.memset(warm, 1.0)
        nc.scalar.activation(warm, warm, FA.Exp)

        # input DMAs, ordered so the (g=0, m=0) matmuls can start earliest:
        # img lhsT m=0 slice, group-0 spec k-chunks 0-1 (all the q=0 matmuls
        # need), chunks 2-3, the rest of img, then groups 1-3 whole.
        nc.sync.dma_start(imgT[:, 0], imgT_d.ap()[:, 0])
        nc.sync.dma_start(specT[:, 0:2, 0:GW], specT_d.ap()[:, 0:2, 0:GW])
        nc.sync.dma_start(specT[:, 2:4, 0:GW], specT_d.ap()[:, 2:4, 0:GW])
        nc.sync.dma_start(imgT[:, 1:T], imgT_d.ap()[:, 1:T])
        for g in range(1, G):
            nc.sync.dma_start(
                specT[:, :, GW * g : GW * (g + 1)],
                specT_d.ap()[:, :, GW * g : GW * (g + 1)],
            )

        # ramp the PE p-state while the input DMAs stream: dummy matmuls on
        # a memset tile keep the tensor clock up so the first real matmuls
        # don't pay the slow-start penalty
        wsrc = pers.tile([P, 64], fp8, name="pewarm")
        nc.vector.memset(wsrc, 0.0)
        for w in range(11):
            wp = ps.tile([P, GW], f32, tag="mm")
            nc.tensor.matmul(
                wp[0:64, 0:64], wsrc, wsrc, start=True, stop=True
            )

        # main loop: logits block, exp, row/col accumulation
        with nc.allow_low_precision("bf16 exp-sum accumulation, <1e-3 on loss"):
            for g in range(G):
                gsl = racc[:, GW * g : GW * (g + 1)]
                for m in range(T):
                    pm = ps.tile([P, GW], f32, tag="mm")
                    # fp8 DoubleRow: each matmul contracts 2 k-chunks (K=256)
                    for q in range(KC // 2):
                        for ns in range(GW // 512):
                            cs = slice(GW * g + 512 * ns, GW * g + 512 * (ns + 1))
                            nc.tensor.matmul(
                                pm[:, 512 * ns : 512 * (ns + 1)],
                                imgT[:, m, 2 * q : 2 * q + 2, :],
                                specT[:, 2 * q : 2 * q + 2, cs],
                                start=(q == 0),
                                stop=(q == KC // 2 - 1),
                                perf_mode=mybir.MatmulPerfMode.DoubleRow,
                            )
                    e = e_tiles[(g * T + m) % 4]
                    nc.scalar.activation(
                        e, pm, FA.Exp,
                        scale=scale / (FP8_PRESCALE * FP8_PRESCALE),
                        accum_out=rowacc[:, m, g : g + 1],
                    )
                    if m == 0:
                        nc.vector.tensor_copy(gsl, e)
                    elif m == T - 1 and g == G - 1:
                        # final tile: halve the add so the write-out overlaps
                        for h in range(2):
                            hs = slice(GW // 2 * h, GW // 2 * (h + 1))
                            nc.vector.tensor_add(
                                out=gsl[:, hs], in0=gsl[:, hs], in1=e[:, hs]
                            )
                            nc.sync.dma_start(
                                racc_o.ap()[:, GW * g + GW // 2 * h :
                                            GW * g + GW // 2 * (h + 1)],
                                gsl[:, hs],
                            )
                    else:
                        nc.vector.tensor_add(out=gsl, in0=gsl, in1=e)
                # racc[g] complete: ship it out now, overlapping next g
                if g != G - 1:
                    nc.sync.dma_start(racc_o.ap()[:, GW * g : GW * (g + 1)], gsl)

        nc.vector.reduce_sum(rows, rowacc[:, :, :], axis=mybir.AxisListType.X)
        nc.sync.dma_start(rowsum_o.ap(), rows)

    nc.compile()
    return nc


def _ensure_ntff_hook():
    """antenv.axon_hooks is absent on this image; provide the tiny get/set
    registry and register trn_agent_boot's ctypes NTFF hook so trace=True
    works. Only used from test runs (KERNEL_TRACE=1)."""
    import sys
    import types

    try:
        import antenv.axon_hooks  # noqa: F401
        return
    except ImportError:
        pass
    mod = types.ModuleType("antenv.axon_hooks")
    _state = {"hook": None}
    mod.set_axon_ntff_profile_hook = lambda h: _state.__setitem__("hook", h)
    mod.get_axon_ntff_profile_hook = lambda: _state["hook"]
    import antenv

    sys.modules["antenv.axon_hooks"] = mod
    antenv.axon_hooks = mod
    try:
        from trn_agent_boot.trn_boot import _ntff_profile_via_ctypes

        mod.set_axon_ntff_profile_hook(
            _ntff_profile_via_ctypes("/opt/axon/libaxon_pjrt.so")
        )
    except Exception as e:  # degrade to no tracing
        print(f"NTFF hook setup failed: {e}")


def kernel(image_features, spectrum_features, logit_scale):
    scale = float(np.asarray(logit_scale))
    key = round(scale, 9)
    if key not in _cache:
        _cache[key] = _build(scale)
    nc = _cache[key]

    import ml_dtypes

    img = np.asarray(image_features, dtype=np.float32)
    spec = np.asarray(spectrum_features, dtype=np.float32)
    imgN = img / np.maximum(
        np.sqrt((img * img).sum(axis=1, keepdims=True)), 1e-3
    )
    specN = spec / np.maximum(
        np.sqrt((spec * spec).sum(axis=1, keepdims=True)), 1e-3
    )
    diag_sum = scale * float(
        np.einsum("nd,nd->", imgN.astype(np.float64), specN.astype(np.float64))
    )

    f8 = ml_dtypes.float8_e4m3fn
    # [p, k, n] = xN[n, 128k + p] * 16 — the PE lhsT/rhs chunk-major layout
    specT8 = np.ascontiguousarray(
        (specN.T * FP8_PRESCALE).astype(f8).reshape(KC, P, N).transpose(1, 0, 2)
    )
    imgT8_all = (imgN.T * FP8_PRESCALE).astype(f8)  # [D, N]
    in_maps = []
    for c in range(C):
        # [p, m, k, j] = imgN[c*NL + 128m + j, 128k + p] * 16
        imgT8 = np.ascontiguousarray(
            imgT8_all[:, c * NL : (c + 1) * NL]
            .reshape(KC, P, T, P)
            .transpose(1, 2, 0, 3)
        )
        in_maps.append({"imgT": imgT8, "specT": specT8})

    trace = os.environ.get("KERNEL_TRACE") == "1"
    if trace:
        _ensure_ntff_hook()
    res = run_bass_kernel_spmd(nc, in_maps, core_ids=list(range(C)), trace=trace)
    if trace:
        print(f"HW exec time: {res.exec_time_ns} ns (mean {res.mean_exec_time_ns})")

    rs = np.stack([r["rowsum"] for r in res.results]).astype(np.float64)  # [C,P,T]
    cs = np.stack(
        [r["racc_o"].astype(np.float64).sum(axis=0) for r in res.results]
    )  # [C,N]

    lse_i_sum = float(np.sum(np.log(rs)))
    lse_s_sum = float(np.sum(np.log(cs.sum(axis=0))))
    loss = 0.5 * ((lse_i_sum - diag_sum) / N + (lse_s_sum - diag_sum) / N)
    return np.float32(loss)
